# revision 6
# baseline (speedup 1.0000x reference)
"""Trainium2 Bass kernel for the NeuralODE Euler-scan problem (v3).

Math reformulation (per core, local batch BL=512 split into 2 blocks of 256):
  reference: x_{t+1} = x_t + dt*(tanh([x_t, I_t] @ W1 + b1) @ W2 + b2)
  we track the pre-activation y_t = x_t @ W1x + I_t*w1i + b1 resident in PSUM:
      h_t     = tanh(y_t)                               (ACT, psum -> sbuf)
      y_{t+1} = y_t + h_t @ (dt*W2@W1x) + dI_t*w1i + dt*b2@W1x   (PE, accum)
  and — new in v3 — the solution x itself is accumulated ON THE PE:
      x_{t+1} = x_t + h_t @ (dt*W2)     (PE matmul, start=False into a
                                         persistent PSUM accumulator)
  so the T-cumsum runs in PSUM f32 for free; each step a DVE tensor_copy
  snapshots the accumulator to an f16 stage tile (the only per-step DVE
  work), and 16-step chunks are DMA'd out as f16.  The host decode is a
  pure layout transform + upcast — no cumsum.

Per-step engine budget (warm, errata cost model):
  ACT: 2 x tanh[128,128]          ~2*(222+128)/1.2  = 584 ns   <- pacing
  PE:  8 matmuls FD=128 + 5 LDW   ~8*56 + 5*30      = 598 ns
  DVE: 2 x copy[32,128] f32->f16  ~2*(120+128)/0.96 = 517 ns
  serial chains (tanh->wzz@h->tanh; w2d->snap->w2d) all < period.

The 256-sample free dim stays split into TWO staggered streams (A = cols
0:128, B = 128:256) with per-stream parity PSUM y banks exactly as v2:
PSUM dep tracking is bank-granular and each tile gets its own bank, so
the only cross-engine gate on the tanh chain is the single wzz@h matmul.
The x accumulators are one PSUM bank per stream; the per-step DVE
snapshot (WAR) serializes w2d(t+1) behind snap(t), a ~370ns cycle that
fits inside the ACT period.

All recurrence matmuls run in float16 (full PE rate at small moving dims;
10-bit mantissa keeps accumulated error ~2e-3, inside the 2e-2 gate).
f16 snapshots of x add < 5e-4 relative — the cumsum itself stays f32 in
PSUM.

Output: (nchunk, 32, 16*256) f16 = 8.4 MB/core (half of v2's f32 deltas),
decoded on host by transpose+astype only.  Batch dim (4096) sharded
across 8 cores; each runs this same program.

di prefetch DMA uses a host-transposed (8, nmmi, S) layout so each group
load is 8 contiguous descriptors instead of 240 strided ones.
"""

import os
import numpy as np

import concourse.bass as bass
from concourse import bacc
import concourse.mybir as mybir
from concourse.tile import TileContext
from concourse import bass_utils

B, T, D, H = 4096, 512, 16, 64
NCORES = 8
BL = B // NCORES          # 512 samples per core
S = BL // 2               # 256 samples per block
HALF = S // 2             # 128 samples per stream
NSTEP = T - 1             # 511 Euler steps
GPF = 30                  # dI prefetch group size (510 = 17*30)
KC = 16                   # steps per output chunk

f32 = mybir.dt.float32
f16 = mybir.dt.float16
TANH = mybir.ActivationFunctionType.Tanh

# column offsets of the constants packed into the single `pack` input
# (each dispatch argument costs ~45us of marshalling over the axon tunnel,
# so the 8 small constants travel as one tensor)
C_WZZ, C_W1I, C_W2D, C_W1X, C_IB, C_ID32, C_X0T, C_I0B, PACK_COLS = (
    0, 128, 256, 288, 416, 544, 576, 832, 1088)


def build_nc(nstep=NSTEP, nchunk=None):
    nmmi = nstep - 1                # number of y-update steps (di8 rows)
    if nchunk is None:
        nchunk = (nstep + KC - 1) // KC
    nc = bacc.Bacc("TRN2", target_bir_lowering=False, debug=False)

    pack_d = nc.dram_tensor("pack", (128, PACK_COLS), f16,
                            kind="ExternalInput")
    di_d = nc.dram_tensor("di", (8, max(nmmi, 1), S), f16, kind="ExternalInput")
    out_d = nc.dram_tensor("xout", (nchunk, 32, KC * S), f16,
                           kind="ExternalOutput")

    with TileContext(nc) as tc:
        with tc.tile_pool(name="consts", bufs=1) as cpool, \
             tc.tile_pool(name="hpool", bufs=4) as hpool, \
             tc.tile_pool(name="dipool", bufs=2) as dipool, \
             tc.tile_pool(name="stpool", bufs=3) as spool, \
             tc.tile_pool(name="ypool", bufs=1, space="PSUM") as ypool, \
             tc.tile_pool(name="xpool", bufs=1, space="PSUM") as xpool:

            packt = cpool.tile([128, PACK_COLS], f16, name="pack_sb")
            nc.sync.dma_start(packt[:, :], pack_d[:, :])
            wzz = packt[:, C_WZZ:C_WZZ + 128]
            w1i = packt[0:8, C_W1I:C_W1I + 128]
            w2d = packt[:, C_W2D:C_W2D + 32]
            w1x = packt[0:32, C_W1X:C_W1X + 128]
            ib = packt[0:4, C_IB:C_IB + 128]
            id32 = packt[0:32, C_ID32:C_ID32 + 32]
            x0t = packt[0:32, C_X0T:C_X0T + S]
            i0b = packt[0:4, C_I0B:C_I0B + S]

            # y state: [parity][stream] -> [128, HALF] psum tile, all
            # initialized to y0 = x0 @ W1x + I0*w1i + b1 (fp32)
            ybank = [[ypool.tile([128, HALF], f32, name=f"y{p}{s}")
                      for s in range(2)] for p in range(2)]
            for p in range(2):
                for s in range(2):
                    nc.tensor.matmul(ybank[p][s][:, :], w1x[:, :],
                                     x0t[:, s * HALF:(s + 1) * HALF],
                                     start=True, stop=False,
                                     skip_group_check=True)
            for p in range(2):
                for s in range(2):
                    nc.tensor.matmul(ybank[p][s][:, :], ib[:, :],
                                     i0b[:, s * HALF:(s + 1) * HALF],
                                     start=False, stop=False,
                                     skip_group_check=True)

            # x accumulators: [32, HALF] psum per stream, init to x0
            xacc = [xpool.tile([32, HALF], f32, name=f"x{s}")
                    for s in range(2)]
            for s in range(2):
                nc.tensor.matmul(xacc[s][:, :], id32[:, :],
                                 x0t[:, s * HALF:(s + 1) * HALF],
                                 start=True, stop=False,
                                 skip_group_check=True)

            di_tiles = {}

            def ensure_di(k, split=0):
                if k in di_tiles or k * GPF >= nmmi:
                    return
                g0 = k * GPF
                gsz = min(GPF, nmmi - g0)
                til = dipool.tile([8, GPF * S], f16, tag="di", name=f"di{k}")
                if split:
                    # fast head so step 0 isn't gated on the full group DMA
                    nc.gpsimd.dma_start(
                        til[:, :split * S].rearrange("p (g s) -> p g s", s=S),
                        di_d[:, g0:g0 + split, :],
                    )
                    nc.sync.dma_start(
                        til[:, split * S:gsz * S].rearrange(
                            "p (g s) -> p g s", s=S),
                        di_d[:, g0 + split:g0 + gsz, :],
                    )
                else:
                    nc.gpsimd.dma_start(
                        til[:, :gsz * S].rearrange("p (g s) -> p g s", s=S),
                        di_d[:, g0:g0 + gsz, :],
                    )
                di_tiles[k] = til

            ensure_di(0, split=10)
            ensure_di(1)

            prev_hA = prev_hB = None
            stage = None
            for t in range(nstep):
                e = t % 2
                u = t % KC
                if u == 0:
                    stage = spool.tile([32, KC * S], f16, tag="stage",
                                       name=f"st{t // KC}")
                    if nstep - t < KC:
                        # partial final chunk: zero-fill so the DMA below
                        # never reads unwritten SBUF
                        nc.any.memset(stage[:, :], 0.0)
                if t % GPF == 0 and t > 0:
                    ensure_di(t // GPF + 1)

                h = hpool.tile([128, S], f16, tag="h", name=f"h{t}")
                hA = h[:, :HALF]
                hB = h[:, HALF:]
                yA, yB = ybank[e]
                last = t >= nstep - 1
                stop = t >= nstep - 3

                # ---- stream A slot ----
                nc.scalar.activation(hA, yA[:, :], TANH)
                if not last:
                    zA, zB = ybank[1 - e]
                    k, s_ = divmod(t, GPF)
                    dA = di_tiles[k][:, s_ * S:s_ * S + HALF]
                    dB = di_tiles[k][:, s_ * S + HALF:(s_ + 1) * S]
                    # off-window updates: run on PE while tanh_A executes
                    nc.tensor.matmul(zA[:, :], w1i[:, :], dA,
                                     start=False, stop=False,
                                     skip_group_check=True)
                    nc.tensor.matmul(zB[:, :], w1i[:, :], dB,
                                     start=False, stop=False,
                                     skip_group_check=True)
                    if t >= 1:
                        nc.tensor.matmul(zA[:, :], wzz[:, :], prev_hA,
                                         start=False, stop=False,
                                         skip_group_check=True)
                        nc.tensor.matmul(zB[:, :], wzz[:, :], prev_hB,
                                         start=False, stop=False,
                                         skip_group_check=True)
                    # window matmul: the only h_A-dependent y update
                    nc.tensor.matmul(zA[:, :], wzz[:, :], hA,
                                     start=False, stop=stop,
                                     skip_group_check=True)
                # x_{t+1} += h_t @ (dt*W2): PSUM cumsum on the PE
                nc.tensor.matmul(xacc[0][:, :], w2d[:, :], hA,
                                 start=False, stop=last,
                                 skip_group_check=True)

                # ---- stream B slot ----
                nc.scalar.activation(hB, yB[:, :], TANH)
                if not last:
                    nc.tensor.matmul(zB[:, :], wzz[:, :], hB,
                                     start=False, stop=stop,
                                     skip_group_check=True)
                nc.tensor.matmul(xacc[1][:, :], w2d[:, :], hB,
                                 start=False, stop=last,
                                 skip_group_check=True)

                # f16 snapshots of x_{t+1} (DVE), packed into the stage tile
                nc.vector.tensor_copy(stage[:, u * S:u * S + HALF],
                                      xacc[0][:, :])
                nc.vector.tensor_copy(stage[:, u * S + HALF:(u + 1) * S],
                                      xacc[1][:, :])

                prev_hA, prev_hB = hA, hB

                if u == KC - 1 or last:
                    nc.sync.dma_start(out_d[t // KC, :, :], stage[:, :])
    nc.compile()
    return nc


def _host_prep(x0, current_profile, tgrid, W1, b1, W2, b2, nstep=NSTEP):
    """Build the shared constants and per-core inputs."""
    nmmi = nstep - 1
    dt = float(np.mean(np.diff(tgrid.astype(np.float64))))
    W1_64 = W1.astype(np.float64)
    W2_64 = W2.astype(np.float64)
    W1x = W1_64[:D]                      # [16, 64]
    w1iv = W1_64[D]                      # [64]
    M = dt * (W2_64 @ W1x)               # [64, 64]
    b2w = dt * (b2.astype(np.float64) @ W1x)   # [64]

    wzz = np.zeros((128, 128), np.float32)
    wzz[:64, :64] = M
    wzz[64:, 64:] = M
    w1i4 = np.zeros((4, 128), np.float32)
    w1i4[0, :64] = w1iv
    w1i4[1, :64] = b2w
    w1i4[2, 64:] = w1iv
    w1i4[3, 64:] = b2w
    w1i8 = np.concatenate([w1i4, w1i4], axis=0)     # [8, 128]
    w2d = np.zeros((128, 32), np.float32)
    w2d[:64, :16] = dt * W2_64
    w2d[64:, 16:] = dt * W2_64
    w1x_blk = np.zeros((32, 128), np.float32)
    w1x_blk[:16, :64] = W1x
    w1x_blk[16:, 64:] = W1x
    ib = np.zeros((4, 128), np.float32)
    ib[0, :64] = w1iv
    ib[1, :64] = b1
    ib[2, 64:] = w1iv
    ib[3, 64:] = b1
    id32 = np.eye(32, dtype=np.float32)
    pack_base = np.zeros((128, PACK_COLS), np.float32)
    pack_base[:, C_WZZ:C_WZZ + 128] = wzz
    pack_base[0:8, C_W1I:C_W1I + 128] = w1i8
    pack_base[:, C_W2D:C_W2D + 32] = w2d
    pack_base[0:32, C_W1X:C_W1X + 128] = w1x_blk
    pack_base[0:4, C_IB:C_IB + 128] = ib
    pack_base[0:32, C_ID32:C_ID32 + 32] = id32

    in_maps = []
    for c in range(NCORES):
        xl = np.asarray(x0[c * BL:(c + 1) * BL], np.float32)     # [512, 16]
        Il = np.asarray(current_profile[c * BL:(c + 1) * BL], np.float32)
        pack = pack_base.copy()
        pack[0:16, C_X0T:C_X0T + S] = xl[:S].T
        pack[16:32, C_X0T:C_X0T + S] = xl[S:].T
        pack[0, C_I0B:C_I0B + S] = Il[:S, 0]
        pack[1, C_I0B:C_I0B + S] = 1.0
        pack[2, C_I0B:C_I0B + S] = Il[S:, 0]
        pack[3, C_I0B:C_I0B + S] = 1.0
        dI = Il[:, 1:nmmi + 1] - Il[:, 0:nmmi]                   # [512, nmmi]
        di4 = np.zeros((max(nmmi, 1), 4, S), np.float32)
        if nmmi:
            di4[:, 0, :] = dI[:S].T
            di4[:, 1, :] = 1.0
            di4[:, 2, :] = dI[S:].T
            di4[:, 3, :] = 1.0
        # di8[t] applies both inc_{t-1}'s and inc_t's input terms: rows 0:4
        # are di4[t-1] (zeros at t=0), rows 4:8 are di4[t]
        di8 = np.zeros((max(nmmi, 1), 8, S), np.float32)
        if nmmi:
            di8[1:, 0:4] = di4[:-1]
            di8[:, 4:8] = di4
        # transpose to (8, nmmi, S) so each prefetch group is a contiguous
        # per-partition DMA slice
        di8t = np.ascontiguousarray(di8.transpose(1, 0, 2)).astype(np.float16)
        in_maps.append(dict(pack=pack.astype(np.float16), di=di8t))
    return dt, in_maps


def _host_decode(arr, xl, dt, b2, nstep=NSTEP):
    """arr: [nchunk, 32, KC*S] f16 x-snapshots for one core -> [BL, nstep+1, D]."""
    nchunk = (nstep + KC - 1) // KC
    a = arr.reshape(nchunk, 2, 16, KC, S)          # (c, b, d, u, s)
    a = a.transpose(1, 4, 0, 3, 2)                 # (b, s, c, u, d)
    # strided astype does the gather + upcast in one pass
    xs = a.astype(np.float32).reshape(BL, nchunk * KC, D)[:, :nstep, :]
    if np.any(b2):
        corr = (np.arange(1, nstep + 1, dtype=np.float64)[:, None]
                * (dt * b2.astype(np.float64))[None, :]).astype(np.float32)
        xs = xs + corr[None, :, :]
    out = np.empty((BL, nstep + 1, D), np.float32)
    out[:, 0] = xl
    out[:, 1:] = xs
    return out


_NC_CACHE = {}


def _get_nc(nstep=NSTEP):
    if nstep not in _NC_CACHE:
        _NC_CACHE[nstep] = build_nc(nstep)
    return _NC_CACHE[nstep]


LAST_RESULTS = None


def kernel(x0, current_profile, t, W1, b1, W2, b2):
    global LAST_RESULTS
    x0 = np.asarray(x0, np.float32)
    current_profile = np.asarray(current_profile, np.float32)
    tgrid = np.asarray(t, np.float32)
    W1 = np.asarray(W1, np.float32)
    b1 = np.asarray(b1, np.float32)
    W2 = np.asarray(W2, np.float32)
    b2 = np.asarray(b2, np.float32)

    dt, in_maps = _host_prep(x0, current_profile, tgrid, W1, b1, W2, b2)
    nc = _get_nc()
    res = bass_utils.run_bass_kernel_spmd(
        nc, in_maps, core_ids=list(range(NCORES)),
        trace=bool(os.environ.get("KERNEL_TRACE")),
    )
    LAST_RESULTS = res

    out = np.empty((B, T, D), np.float32)
    for c in range(NCORES):
        xl = x0[c * BL:(c + 1) * BL]
        out[c * BL:(c + 1) * BL] = _host_decode(
            res.results[c]["xout"], xl, dt, b2)
    return out
